# revision 25
# baseline (speedup 1.0000x reference)
"""GQA attention (dense_transformer) on 8 TRN2 NeuronCores — v5.

Sharding: 2-way data parallel over batch x 4-way tensor parallel over
heads. Core c owns batch c//4 and q-heads {4t..4t+3}, t = c%4, plus kv
head t (exactly one kv group per core -> zero duplicated k/v work).
Wq/Wk/Wv column-parallel, Wo row-parallel; the 4 partial o_proj outputs
per batch are summed on the host.

Per-core schedule (all matmuls fp16; fp8 rejected: for random-sign
contractions its ~2% quantization error does not average down and blows
the 2e-2 budget):
  - Phase A: per 512-token block, k+v projections (2x2 PSUM banks,
    double buffered); for blocks 0,1 the four q-head projections are
    fused in (4 more banks) so x^T is streamed from HBM exactly once
    and the whole phase stays PE-bound (~100 GB/s). k gets RoPE'd into
    kr, V^T -> V via DMA xbar transposes. Blocks 2,3 stay resident in
    the stage pool for phase B.
  - Phase B: 8 paired cells [q-head pass over resident block 2/3 +
    attention unit for sq-groups 0/2] then 8 solo units (sq-groups 1/3)
    with o_proj chunks woven in. Every cell carries ~10.4us of PE work
    against ~9us of ACT exp, so the softmax stream never throttles PE.
  - Attention unit: scores into 2-bank PSUM groups, exp on ACT at
    free-dim 1024 into fp16 ex tiles, AV matmuls software-pipelined
    with lag-2 (carried across unit boundaries). The softmax epilogue
    (DVE fold tree in half-sized pieces, denominator ones-matmul,
    reciprocal, normalize) rides slot-by-slot inside the NEXT unit so
    the in-order PE queue never waits on it. out_ps/sums alternate
    between two parity-tagged PSUM banks to make that safe.
  - o_proj: 1-bank chunks, 4 per retired sq-group, woven at unit slot
    g7 + a short drain at the end; copies alternate ACT/DVE; output
    DMAs issue from GpSimd so no copy engine ever blocks a trigger.
"""

import math

import numpy as np

import concourse.bacc as bacc_mod
import concourse.mybir as mybir
import concourse.tile as tile
from concourse.bass_utils import run_bass_kernel_spmd

HIDDEN = 2048
N_HEADS = 16
N_KV_HEADS = 4
HEAD_DIM = 128
ROPE_THETA = 10000.0
B = 2
S = 2048
N_CORES = 8
TP = 4  # tensor-parallel ways per batch
NH_LOC = N_HEADS // TP  # 4 q heads per core
P = 128
KT = HIDDEN // P  # 16 contraction k-tiles over hidden
NSK = S // P  # 16 sk tiles
NB = S // 512  # 4 token blocks of 512
NSQG = S // 512  # 4 sq groups of 512
F32 = mybir.dt.float32
FP16 = mybir.dt.float16
SCALE = 1.0 / math.sqrt(HEAD_DIM)


def _rope_tables(s, d):
    inv_freq = 1.0 / (ROPE_THETA ** (np.arange(0, d, 2, dtype=np.float32) / d))
    t = np.arange(s, dtype=np.float32)
    freqs = np.outer(t, inv_freq).astype(np.float32)  # [S, d/2]
    emb = np.concatenate([freqs, freqs], axis=-1)  # [S, d]
    cos_t = np.ascontiguousarray(np.cos(emb).T)  # [d, S]
    # rotate_half sign folded into sin rows: rows 0..63 multiply -x[64:128]
    sin_t = np.sin(emb).T.copy()
    sin_t[: d // 2, :] *= -1.0
    return cos_t.astype(np.float16), np.ascontiguousarray(sin_t).astype(np.float16)


def _ktile(a, p=P):
    """[K, M] -> [p, K//p, M] with contraction index = tile*p + partition."""
    k, m = a.shape
    return np.ascontiguousarray(a.reshape(k // p, p, m).transpose(1, 0, 2))


def _build(add_mask):
    nc = bacc_mod.Bacc()
    xt_d = nc.dram_tensor("xt", [P, KT, S], FP16, kind="ExternalInput")
    wq_d = nc.dram_tensor("wq", [P, KT, NH_LOC * P], FP16, kind="ExternalInput")
    wk_d = nc.dram_tensor("wk", [P, KT, P], FP16, kind="ExternalInput")
    wv_d = nc.dram_tensor("wv", [P, KT, P], FP16, kind="ExternalInput")
    wo_d = nc.dram_tensor("wo", [P, NH_LOC, HIDDEN], FP16, kind="ExternalInput")
    cos_d = nc.dram_tensor("cos_t", [P, S], FP16, kind="ExternalInput")
    sin_d = nc.dram_tensor("sin_t", [P, S], FP16, kind="ExternalInput")
    if add_mask:
        # mask transposed + k-tiled: [P, NSK, S] ([sk%P, sk//P, sq])
        mt_d = nc.dram_tensor("mask_t", [P, NSK, S], F32, kind="ExternalInput")
    out_d = nc.dram_tensor("out", [S, HIDDEN], FP16, kind="ExternalOutput")

    h = P // 2

    with tile.TileContext(nc) as tc:
        with (
            tc.tile_pool(name="consts", bufs=1) as consts,
            tc.tile_pool(name="persist", bufs=1) as persist,
        ):
            wk_sb = consts.tile([P, KT, P], FP16, tag="wk")
            wv_sb = consts.tile([P, KT, P], FP16, tag="wv")
            wq_sb = consts.tile([P, KT, NH_LOC * P], FP16, tag="wq")
            wo_sb = consts.tile([P, NH_LOC, HIDDEN], FP16, tag="wo")
            cos_sb = consts.tile([P, S], FP16, tag="cos")
            sin_sb = consts.tile([P, S], FP16, tag="sin")
            ones_sb = consts.tile([P, P], FP16, tag="ones")
            # PE-gating DMAs lead: kv weights, then wq k-tile chunks (the
            # fused q projections consume them progressively). cos/sin
            # only gate the DVE RoPE chain, never PE — they ride behind.
            nc.sync.dma_start(out=wk_sb, in_=wk_d[:, :, :])
            nc.sync.dma_start(out=wv_sb, in_=wv_d[:, :, :])
            for cchunk in range(4):
                csl = slice(4 * cchunk, 4 * cchunk + 4)
                nc.scalar.dma_start(out=wq_sb[:, csl, :], in_=wq_d[:, csl, :])
            nc.scalar.dma_start(out=cos_sb, in_=cos_d[:, :])
            nc.scalar.dma_start(out=sin_sb, in_=sin_d[:, :])
            nc.gpsimd.memset(ones_sb, 1.0)
            # warm the ACT exp table before the softmax stream
            scr = consts.tile([P, 16], FP16, tag="scr")
            nc.scalar.activation(
                scr, cos_sb[:, 0:16], mybir.ActivationFunctionType.Exp)

            qr = [
                persist.tile([P, S], FP16, tag=f"qr{m}", name=f"qr{m}")
                for m in range(NH_LOC)
            ]
            kr = persist.tile([P, S], FP16, tag="kr", name="kr")
            vn = persist.tile([P, NSK, P], FP16, tag="vn", name="vn")
            outn = persist.tile([P, NH_LOC, S], FP16, tag="outn", name="outn")

            st_ctx = tc.tile_pool(name="stage", bufs=1)
            st = st_ctx.__enter__()  # closed LIFO before TileContext exit

            def rope(src, dst_sl_tensor, sl, eng_a, eng_b):
                """rotate-half via partition-swap DMA; sign folded in sin."""
                tq = st.tile([P, 512], FP16, tag="tq", bufs=4, name="tq")
                eng_a.dma_start(out=tq[0:h, :], in_=src[h:P, :])
                eng_b.dma_start(out=tq[h:P, :], in_=src[0:h, :])
                tcs = st.tile([P, 512], FP16, tag="tcs", bufs=4, name="tcs")
                nc.vector.tensor_mul(tcs, src, cos_sb[:, sl])
                nc.vector.tensor_mul(tq, tq, sin_sb[:, sl])
                nc.vector.tensor_add(dst_sl_tensor, tcs, tq)

            def emit_qhead(xt_sb, ppq, sl, m):
                for c in range(KT):
                    nc.tensor.matmul(
                        ppq, wq_sb[:, c, m * P : (m + 1) * P],
                        xt_sb[:, c, :], start=c == 0, stop=c == KT - 1)
                q_st = st.tile([P, 512], FP16, tag="qst", bufs=2,
                               name="q_st")
                nc.scalar.copy(q_st, ppq)
                rope(q_st, qr[m][:, sl], sl, nc.gpsimd, nc.scalar)

            # ------- Phase A: projections (q fused in for blocks 0,1) -----
            xt_tiles = {}
            with tc.tile_pool(name="ps_a", bufs=1, space="PSUM") as pkv:
                for blk in range(NB):
                    sl = slice(blk * 512, (blk + 1) * 512)
                    xt_sb = st.tile([P, KT, 512], FP16, tag="xt", bufs=2,
                                    name="xt_sb")
                    xt_tiles[blk] = xt_sb
                    nc.gpsimd.dma_start(out=xt_sb[:, 0:4, :],
                                        in_=xt_d[:, 0:4, sl])
                    nc.gpsimd.dma_start(out=xt_sb[:, 4:8, :],
                                        in_=xt_d[:, 4:8, sl])
                    nc.sync.dma_start(out=xt_sb[:, 8:12, :],
                                      in_=xt_d[:, 8:12, sl])
                    nc.sync.dma_start(out=xt_sb[:, 12:KT, :],
                                      in_=xt_d[:, 12:KT, sl])
                    ppkv = pkv.tile([P, 2, 512], F32, tag="ppkv", bufs=2,
                                    name="ppkv")
                    for c in range(KT):
                        st_ = c == 0
                        sp_ = c == KT - 1
                        nc.tensor.matmul(ppkv[:, 0, :], wk_sb[:, c, :],
                                         xt_sb[:, c, :], start=st_, stop=sp_)
                        nc.tensor.matmul(ppkv[:, 1, :], wv_sb[:, c, :],
                                         xt_sb[:, c, :], start=st_, stop=sp_)
                    k_st = st.tile([P, 512], FP16, tag="kst", bufs=2,
                                   name="k_st")
                    vt_st = st.tile([P, 512], FP16, tag="vst", bufs=2,
                                    name="vt_st")
                    nc.scalar.copy(k_st, ppkv[:, 0, :])
                    nc.scalar.copy(vt_st, ppkv[:, 1, :])
                    rope(k_st, kr[:, sl], sl, nc.scalar, nc.gpsimd)
                    for j in range(4):
                        nc.sync.dma_start_transpose(
                            vn[:, 4 * blk + j, :], vt_st[:, j * P : (j + 1) * P])
                    if blk < 2:
                        # fused q projections: x is in SBUF right now, so
                        # spend the PE time while the next block's DMA runs
                        ppq4 = pkv.tile([P, NH_LOC, 512], F32, tag="ppq4",
                                        bufs=1, name="ppq4")
                        for m in range(NH_LOC):
                            emit_qhead(xt_sb, ppq4[:, m, :], sl, m)
            # o_proj weights aren't needed until the first drain chunks
            nc.scalar.dma_start(out=wo_sb, in_=wo_d[:, :, :])

            # ------- Phase B: q passes (blocks 2,3) + attention units -----
            ex_prev = None  # dict state of the previous unit
            av_pend = []  # software-pipeline queue of pending AV matmuls
            po_pend = []  # o_proj chunks ready to emit (sqt, hc)
            ob_tiles = {}  # sqt -> ob staging tile
            ucount = [0]

            with tc.tile_pool(name="stage_b", bufs=1) as sb:

                def emit_av(ex, t, out_ps):
                    nc.tensor.matmul(out_ps, vn[:, t, :], ex[:, t, :],
                                     start=t == 0, stop=t == NSK - 1)

                def pop_av(n=1):
                    for _ in range(n):
                        if av_pend:
                            emit_av(*av_pend.pop(0))

                def emit_po_chunk(ppo):
                    sqt, hc = po_pend.pop(0)
                    po = ppo.tile([P, 512], F32, tag="po", bufs=2, name="po")
                    for dc in range(NH_LOC):
                        nc.tensor.matmul(
                            po, outn[:, dc, sqt * P : (sqt + 1) * P],
                            wo_sb[:, dc, hc * 512 : (hc + 1) * 512],
                            start=dc == 0, stop=dc == NH_LOC - 1)
                    if sqt not in ob_tiles:
                        ob_tiles[sqt] = sb.tile([P, HIDDEN], FP16, tag="ob",
                                                bufs=3, name="ob")
                    ob = ob_tiles[sqt]
                    eng = nc.vector.tensor_copy if hc & 1 else nc.scalar.copy
                    eng(ob[:, hc * 512 : (hc + 1) * 512], po)
                    if hc == 3:
                        nc.gpsimd.dma_start(
                            out=out_d[sqt * P : (sqt + 1) * P, :], in_=ob)
                        del ob_tiles[sqt]

                def emit_unit(sqg, m, pb, ppo):
                    """One attention unit, with the previous unit's epilogue
                    and up to four o_proj chunks woven into its slots.

                    out_ps/sums alternate between two parity-tagged PSUM
                    banks: unit u's accumulator must survive until its onr
                    copy, which is emitted during unit u+1 — after u+1's
                    first AV write. Parity keeps u+1 off u's bank, and the
                    denominator matmul then reuses u's bank (its writer
                    follows the onr read in program order)."""
                    nonlocal ex_prev
                    uidx = ucount[0]
                    ucount[0] += 1
                    qsl = slice(sqg * 512, (sqg + 1) * 512)
                    ex = sb.tile([P, NSK, 512], FP16, tag="ex", bufs=2,
                                 name="ex")
                    out_ps = pb.tile([P, 512], F32, tag=f"out{uidx & 1}",
                                     bufs=1, name="out_ps")
                    pe = ex_prev
                    for g in range(8):
                        scg = pb.tile([P, 2, 512], F32, tag="scg", bufs=2,
                                      name="scg")
                        for j in range(2):
                            t = 2 * g + j
                            nc.tensor.matmul(
                                scg[:, j, :], kr[:, t * P : (t + 1) * P],
                                qr[m][:, qsl], start=True, stop=True)
                        if add_mask:
                            mk = sb.tile([P, 2, 512], F32, tag="mk", bufs=4,
                                         name="mk")
                            nc.sync.dma_start(
                                out=mk, in_=mt_d[:, 2 * g : 2 * g + 2, qsl])
                            nc.vector.scalar_tensor_tensor(
                                scg, scg, SCALE, mk,
                                op0=mybir.AluOpType.mult,
                                op1=mybir.AluOpType.add)
                            nc.scalar.activation(
                                ex[:, 2 * g : 2 * g + 2, :], scg,
                                mybir.ActivationFunctionType.Exp)
                        else:
                            nc.scalar.activation(
                                ex[:, 2 * g : 2 * g + 2, :], scg,
                                mybir.ActivationFunctionType.Exp,
                                scale=SCALE)
                        av_pend.append((ex, 2 * g, out_ps))
                        av_pend.append((ex, 2 * g + 1, out_ps))
                        pop_av(2 if len(av_pend) > 4 else 1)
                        # previous unit's epilogue rides this unit's slots;
                        # the fold tree runs on DVE in half-sized pieces so
                        # the sums matmul (g6, PE in-order!) never blocks.
                        if pe is not None:
                            pex = pe["ex"]
                            if g == 0:
                                nc.vector.tensor_add(
                                    pex[:, 0:4, :], pex[:, 0:4, :],
                                    pex[:, 8:12, :])
                            elif g == 1:
                                nc.vector.tensor_add(
                                    pex[:, 4:8, :], pex[:, 4:8, :],
                                    pex[:, 12:16, :])
                            elif g == 2:
                                nc.vector.tensor_add(
                                    pex[:, 0:2, :], pex[:, 0:2, :],
                                    pex[:, 4:6, :])
                            elif g == 3:
                                nc.vector.tensor_add(
                                    pex[:, 2:4, :], pex[:, 2:4, :],
                                    pex[:, 6:8, :])
                            elif g == 4:
                                nc.vector.tensor_add(
                                    pex[:, 0:2, :], pex[:, 0:2, :],
                                    pex[:, 2:4, :])
                                pe["onr"] = sb.tile([P, 512], F32,
                                                    tag="onr", bufs=2,
                                                    name="onr")
                                nc.scalar.copy(pe["onr"], pe["out"])
                            elif g == 5:
                                nc.vector.tensor_add(
                                    pex[:, 0, :], pex[:, 0, :], pex[:, 1, :])
                            elif g == 6:
                                pe["sums"] = pb.tile(
                                    [P, 512], F32, tag=f"out{pe['idx'] & 1}",
                                    bufs=1, name="sums")
                                nc.tensor.matmul(pe["sums"], ones_sb,
                                                 pex[:, 0, :],
                                                 start=True, stop=True)
                            elif g == 7:
                                rec = sb.tile([P, 512], F32, tag="rec",
                                              bufs=2, name="rec")
                                nc.vector.reciprocal_approx_fast(
                                    rec, pe["sums"])
                                nc.vector.tensor_mul(
                                    outn[:, pe["m"], pe["qsl"]],
                                    pe["onr"], rec)
                                if pe["m"] == NH_LOC - 1:
                                    # that sq-group's outn is now complete
                                    for sqt in range(4 * pe["sqg"],
                                                     4 * pe["sqg"] + 4):
                                        for hc in range(4):
                                            po_pend.append((sqt, hc))
                                if ppo is not None:
                                    for _ in range(4):
                                        if po_pend:
                                            emit_po_chunk(ppo)
                    ex_prev = dict(ex=ex, out=out_ps, m=m, qsl=qsl,
                                   idx=uidx, sqg=sqg)

                def emit_epilogue_final(pb):
                    """Flush the last unit's epilogue serially."""
                    pop_av(len(av_pend))
                    pe = ex_prev
                    pex = pe["ex"]
                    onr = sb.tile([P, 512], F32, tag="onr", bufs=2,
                                  name="onr")
                    nc.scalar.copy(onr, pe["out"])
                    nc.vector.tensor_add(pex[:, 0:8, :], pex[:, 0:8, :],
                                         pex[:, 8:16, :])
                    nc.vector.tensor_add(pex[:, 0:4, :], pex[:, 0:4, :],
                                         pex[:, 4:8, :])
                    nc.vector.tensor_add(pex[:, 0:2, :], pex[:, 0:2, :],
                                         pex[:, 2:4, :])
                    nc.vector.tensor_add(pex[:, 0, :], pex[:, 0, :],
                                         pex[:, 1, :])
                    sums = pb.tile([P, 512], F32, tag=f"out{pe['idx'] & 1}",
                                   bufs=1, name="sums")
                    nc.tensor.matmul(sums, ones_sb, pex[:, 0, :],
                                     start=True, stop=True)
                    rec = sb.tile([P, 512], F32, tag="rec", bufs=2,
                                  name="rec")
                    nc.vector.reciprocal_approx_fast(rec, sums)
                    nc.vector.tensor_mul(outn[:, pe["m"], pe["qsl"]],
                                         onr, rec)
                    for sqt in range(4 * pe["sqg"], 4 * pe["sqg"] + 4):
                        for hc in range(4):
                            po_pend.append((sqt, hc))

                with tc.tile_pool(name="ps_b", bufs=1, space="PSUM") as pb:
                    # paired cells: q-head pass over resident block 2/3 +
                    # one unit whose q slice is already projected
                    with tc.tile_pool(name="ps_q", bufs=1,
                                      space="PSUM") as pq:
                        for blk, usqg in ((2, 0), (3, 2)):
                            sl = slice(blk * 512, (blk + 1) * 512)
                            for m in range(NH_LOC):
                                ppq = pq.tile([P, 512], F32, tag="ppq",
                                              bufs=2, name="ppq")
                                emit_qhead(xt_tiles[blk], ppq, sl, m)
                                emit_unit(usqg, m, pb, None)
                    # solo units for sq-groups 1,3 + woven o_proj
                    with tc.tile_pool(name="ps_o", bufs=1,
                                      space="PSUM") as ppo:
                        for sqg in (1, 3):
                            for m in range(NH_LOC):
                                emit_unit(sqg, m, pb, ppo)
                        emit_epilogue_final(pb)
                        while po_pend:
                            emit_po_chunk(ppo)
            st_ctx.__exit__(None, None, None)
    nc.compile()
    return nc


_BUILD_CACHE = {}
LAST_RESULT = None


def _get_nc(add_mask):
    if add_mask not in _BUILD_CACHE:
        _BUILD_CACHE[add_mask] = _build(add_mask)
    return _BUILD_CACHE[add_mask]


def kernel(hidden_states, attention_mask, Wq, Wk, Wv, Wo):
    hidden_states = np.asarray(hidden_states, dtype=np.float32)
    attention_mask = np.asarray(attention_mask, dtype=np.float32)
    Wq = np.asarray(Wq, dtype=np.float32)
    Wk = np.asarray(Wk, dtype=np.float32)
    Wv = np.asarray(Wv, dtype=np.float32)
    Wo = np.asarray(Wo, dtype=np.float32)

    b, s, hidden = hidden_states.shape
    assert (b, s, hidden) == (B, S, HIDDEN)

    add_mask = bool(np.any(attention_mask))
    nc = _get_nc(add_mask)

    xts = [
        _ktile(hidden_states[bi].T.astype(np.float16)) for bi in range(B)
    ]
    cos_t, sin_t = _rope_tables(s, HEAD_DIM)
    if add_mask:
        mt_kt = _ktile(np.ascontiguousarray(attention_mask[0, 0].T))

    wqs, wks, wvs, wos = [], [], [], []
    for tp in range(TP):
        wqs.append(_ktile(
            Wq[:, tp * NH_LOC * HEAD_DIM : (tp + 1) * NH_LOC * HEAD_DIM]
            .astype(np.float16)))
        wks.append(_ktile(
            Wk[:, tp * HEAD_DIM : (tp + 1) * HEAD_DIM].astype(np.float16)))
        wvs.append(_ktile(
            Wv[:, tp * HEAD_DIM : (tp + 1) * HEAD_DIM].astype(np.float16)))
        wos.append(np.ascontiguousarray(
            Wo[tp * NH_LOC * HEAD_DIM : (tp + 1) * NH_LOC * HEAD_DIM, :]
            .astype(np.float16)
            .reshape(NH_LOC, P, HIDDEN).transpose(1, 0, 2)))

    in_maps = []
    for c in range(N_CORES):
        bi, tp = divmod(c, TP)
        im = {
            "xt": xts[bi],
            "cos_t": cos_t,
            "sin_t": sin_t,
            "wq": wqs[tp],
            "wk": wks[tp],
            "wv": wvs[tp],
            "wo": wos[tp],
        }
        if add_mask:
            im["mask_t"] = mt_kt
        in_maps.append(im)

    res = run_bass_kernel_spmd(nc, in_maps, core_ids=list(range(N_CORES)))
    global LAST_RESULT
    LAST_RESULT = res
    out = np.zeros((b, s, hidden), dtype=np.float32)
    for c, r in enumerate(res.results):
        bi = c // TP
        out[bi] += np.asarray(r["out"], dtype=np.float32)
    return out.reshape(b, s, hidden)
